# revision 18
# baseline (speedup 1.0000x reference)
"""Trainium2 kernel for BinaryLinear: out = x @ sign(clip(weight,-1,1)).T + bias.

Full shapes: x [8192, 4096] f32, weight [4096, 4096] f32, bias [4096] f32,
out [8192, 4096] f32. 8 NeuronCores, no collectives needed.

Design (measured on HW via neuron-profile):
  - Grid-shard tokens x out_features (2x4) across the 8 cores; each core
    computes a disjoint [4096, 1024] output tile; host slices inputs /
    stitches outputs.
  - Binarized weights are exactly +-1 (bf16-exact). x ships as bf16
    (~1.2e-3 rel error, gate is 2e-2). bf16 is the fastest matmul path:
    512-row moving streams at 1 col/cycle (216ns/matmul issue cadence =
    512 cycles @2.4GHz + 2.5ns NX dispatch). f32r pays a 187ns LDWEIGHTS
    that gates the cadence at 227ns; fp16 streams slower (235ns); fp8
    DoubleRow is only ~1.44x and would need hi/lo two passes (net loss).
    512 moving columns is the ISA max per matmul (s3d3_mm_num_elements).
  - Host packs x transposed+tiled so the contraction dim (in_features)
    lands on SBUF partitions with every DMA contiguous at line rate.
  - The PE busy floor is 2048 matmuls x 216ns = 442us; everything else
    is startup/stream/tail engineering:
    * Weights + first x chunks are woven across BOTH HW DGE queues
      (SP + Activation, ~170GB/s each) in first-need order with a greedy
      balance, in 256KB items. (Halving item granularity to 128KB was
      tried and is ~5us SLOWER overall: the doubled descriptor-write
      count puts recurring micro-stalls into the steady-state stream.)
    * Trickle phase: 3 token blocks round-robin over kb pairs consume
      weight k-blocks at 1.3us/kb vs ~1.1us/kb dual-queue arrival, so
      the weight slice becomes SBUF-resident with only ~3us of gaps.
    * Steady state: n-major per block; each PSUM bank is flushed
      (DVE adds bias while copying PSUM->SBUF) and DMA'd out the moment
      its accumulation closes, alternating output queues; x prefetch
      stays 2 blocks ahead on the SP queue.
    * The final bank flushes in two 256-col halves racing down both DMA
      queues to shorten the kernel tail.
  - Total ~462-470us (2.4GHz power mood): ~10us fixed runtime/DMA-start
    latency + 442us PE-bound compute + ~3us gaps + ~6us tail. The chip
    sometimes sits at a 2.0GHz package power state (~552us) - cadence
    259ns - which no kernel-side change can affect.
"""

import sys

if "/opt/trn_rl_repo" not in sys.path:
    sys.path.insert(0, "/opt/trn_rl_repo")

import ml_dtypes
import numpy as np

N_TOK, D_IN, D_OUT = 8192, 4096, 4096
TOK_SHARDS, OUT_SHARDS = 2, 4
N_CORES = TOK_SHARDS * OUT_SHARDS
TOK_C = N_TOK // TOK_SHARDS
OUT_C = D_OUT // OUT_SHARDS
MB = TOK_C // 128  # token blocks per core
KB = D_IN // 128  # contraction blocks
NF = 512  # matmul moving free dim (one fp32 PSUM bank, ISA max)
NB = OUT_C // NF  # PSUM banks per token block
XCH = 4  # x chunks per trickle block (8 kbs / 256KB each)
TRICKLE = 3  # token blocks interleaved with the weight stream at startup

_cached_nc = None


def build_nc():
    import concourse.bacc as bacc
    import concourse.mybir as mybir
    import concourse.tile as tile

    dt = mybir.dt
    mdt = dt.bfloat16

    nc = bacc.Bacc()
    xh_d = nc.dram_tensor("xh", [MB, 128, D_IN], mdt, kind="ExternalInput")
    wt_d = nc.dram_tensor("wt", [KB, 128, OUT_C], dt.float8e4, kind="ExternalInput")
    br_d = nc.dram_tensor("br", [128, OUT_C], dt.float32, kind="ExternalInput")
    out_d = nc.dram_tensor("out", [TOK_C, OUT_C], dt.float32, kind="ExternalOutput")

    with tile.TileContext(nc) as tc:
        with (
            tc.tile_pool(name="wts", bufs=1) as wpool,
            tc.tile_pool(name="wstage", bufs=4) as spool,
            tc.tile_pool(name="bias", bufs=1) as bpool,
            tc.tile_pool(name="xin", bufs=6) as xpool,
            tc.tile_pool(name="outp", bufs=2) as opool,
            tc.tile_pool(name="psum", bufs=8, space="PSUM") as ppool,
        ):
            # --- greedy dual-queue DMA weave -------------------------------
            qclock = {"sync": 0.0, "scalar": 0.0}
            ITEM_US = 1.5  # ~256KB at ~170GB/s per active queue

            def enq(cost_units=1.0):
                q = min(qclock, key=qclock.get)
                qclock[q] += cost_units * ITEM_US
                return getattr(nc, q)

            wts = {}

            def load_w(kb):
                # +-1 is fp8-exact: ship 1 byte/weight (halves the startup
                # weight stream to 4MB) and upcast on the mostly-idle DVE.
                stage = spool.tile(
                    [128, OUT_C], dt.float8e4, name=f"ws{kb}", tag="wstage"
                )
                enq(0.5).dma_start(stage[:], wt_d[kb])
                w = wpool.tile([128, OUT_C], mdt, name=f"wt{kb}", tag=f"wt{kb}")
                nc.vector.tensor_copy(w[:], stage[:])
                wts[kb] = w

            xt = {}
            xc_done = {}

            def x_tile(m):
                x = xpool.tile([128, D_IN], mdt, name=f"xh_{m}", tag="xh")
                xt[m] = x
                xc_done[m] = 0
                return x

            def load_x_chunk(m):
                c = xc_done[m]
                cw = D_IN // XCH
                lo, hi = c * cw, (c + 1) * cw
                enq().dma_start(xt[m][:, lo:hi], xh_d[m][:, lo:hi])
                xc_done[m] = c + 1

            def load_x_full(m, engine):
                x = x_tile(m)
                engine.dma_start(x[:], xh_d[m])
                xc_done[m] = XCH

            # Trickle DMA stream, first-need order with a one-chunk x lead.
            for m in range(TRICKLE):
                x_tile(m)
            load_x_chunk(0)
            load_w(0)
            load_w(1)
            load_x_chunk(1)
            load_x_chunk(2)
            for k0 in range(2, KB, 2):
                # request x chunk c when entering kb region 8c-4 (lead 4 kbs)
                if (k0 + 4) % 8 == 0:
                    for m in range(TRICKLE):
                        if xc_done[m] < XCH:
                            load_x_chunk(m)
                load_w(k0)
                load_w(k0 + 1)
            bias_s = bpool.tile([128, OUT_C], dt.float32, name="bias_s")
            enq(2.0).dma_start(bias_s[:], br_d[:])
            load_x_full(3, nc.sync)
            load_x_full(4, nc.sync)

            # --- compute ---------------------------------------------------
            def alloc_ps(m):
                return [
                    ppool.tile([128, NF], dt.float32, name=f"ps_{m}_{n}", tag="ps")
                    for n in range(NB)
                ]

            def mm(m, kb, n, ps):
                nc.tensor.matmul(
                    ps[n][:],
                    xt[m][:, kb * 128 : (kb + 1) * 128],
                    wts[kb][:, n * NF : (n + 1) * NF],
                    start=(kb == 0),
                    stop=(kb == KB - 1),
                )

            def flush_bank(m, n, ps, out_t, split=1):
                fw = NF // split
                for s in range(split):
                    lo, hi = n * NF + s * fw, n * NF + (s + 1) * fw
                    nc.vector.tensor_tensor(
                        out_t[:, lo:hi],
                        ps[n][:, s * fw : (s + 1) * fw],
                        bias_s[:, lo:hi],
                        mybir.AluOpType.add,
                    )
                    eng = nc.scalar if s % 2 == 0 else nc.sync
                    eng.dma_start(
                        out_d[m * 128 : (m + 1) * 128, lo:hi], out_t[:, lo:hi]
                    )

            # Trickle: 3 token blocks round-robin over kb pairs. Weight-kb
            # consumption 1.3us/kb vs dual-queue stream arrival ~1.1us/kb
            # (weights + x chunks + bias woven by first need), so the PE
            # tracks the stream with only pipeline-fill gaps at the start.
            tps = [alloc_ps(m) for m in range(TRICKLE)]
            for k0 in range(0, KB, 2):
                for m in range(TRICKLE):
                    for kb in (k0, k0 + 1):
                        for n in range(NB):
                            mm(m, kb, n, tps[m])
            for m in range(TRICKLE):
                out_t = opool.tile([128, OUT_C], dt.float32, name=f"o_{m}", tag="out")
                for n in range(NB):
                    flush_bank(m, n, tps[m], out_t)

            # Steady state: n-major per block, flush+store each PSUM bank as
            # soon as its accumulation closes. x prefetch stays 3 blocks out.
            for m in range(TRICKLE, MB):
                if m + 2 < MB:
                    load_x_full(m + 2, nc.sync)
                ps = alloc_ps(m)
                out_t = opool.tile([128, OUT_C], dt.float32, name=f"o_{m}", tag="out")
                last = m == MB - 1
                for n in range(NB):
                    if last and n == NB - 1:
                        # two independent 256-col accumulation groups in one
                        # PSUM bank: the left half closes a ~3.5us sweep early,
                        # so its flush+store overlap the right half's matmuls
                        # and only a 128KB store trails the final matmul.
                        for h in range(2):
                            cl = n * NF + h * 256
                            ch = cl + 256
                            pl, ph = h * 256, (h + 1) * 256
                            for kb in range(KB):
                                nc.tensor.matmul(
                                    ps[n][:, pl:ph],
                                    xt[m][:, kb * 128 : (kb + 1) * 128],
                                    wts[kb][:, cl:ch],
                                    start=(kb == 0),
                                    stop=(kb == KB - 1),
                                )
                            nc.vector.tensor_tensor(
                                out_t[:, cl:ch],
                                ps[n][:, pl:ph],
                                bias_s[:, cl:ch],
                                mybir.AluOpType.add,
                            )
                            eng = nc.scalar if h == 0 else nc.sync
                            eng.dma_start(
                                out_d[m * 128 : (m + 1) * 128, cl:ch],
                                out_t[:, cl:ch],
                            )
                    else:
                        for kb in range(KB):
                            mm(m, kb, n, ps)
                        flush_bank(m, n, ps, out_t)

    nc.compile()
    return nc


def _pack_x(a):
    """[TOK_C, D_IN] -> [MB, 128, D_IN] with layout [m, p, (kb t)]:
    packed[m, p, kb*128 + t] = a[m*128 + t, kb*128 + p]."""
    return np.ascontiguousarray(
        a.reshape(MB, 128, KB, 128).transpose(0, 3, 2, 1)
    ).reshape(MB, 128, D_IN)


def prepare_in_maps(x, weight, bias):
    x = np.asarray(x, dtype=np.float32)
    weight = np.asarray(weight, dtype=np.float32)
    bias = np.asarray(bias, dtype=np.float32)

    bw = np.where(weight >= 0, np.float32(1.0), np.float32(-1.0))

    wt_packs, bias_packs = [], []
    for oi in range(OUT_SHARDS):
        w_sh = bw[oi * OUT_C : (oi + 1) * OUT_C]  # [OUT_C, D_IN]
        wt = np.ascontiguousarray(w_sh.T).astype(ml_dtypes.float8_e4m3)
        wt_packs.append(wt.reshape(KB, 128, OUT_C))
        bias_packs.append(
            np.ascontiguousarray(
                np.broadcast_to(bias[oi * OUT_C : (oi + 1) * OUT_C], (128, OUT_C))
            )
        )

    xh_packs = []
    for ti in range(TOK_SHARDS):
        x_sh = x[ti * TOK_C : (ti + 1) * TOK_C].astype(ml_dtypes.bfloat16)
        xh_packs.append(_pack_x(x_sh))

    in_maps = []
    for c in range(N_CORES):
        ti, oi = divmod(c, OUT_SHARDS)
        in_maps.append(
            {"xh": xh_packs[ti], "wt": wt_packs[oi], "br": bias_packs[oi]}
        )
    return in_maps


def run(in_maps, trace=False, **kwargs):
    global _cached_nc
    from concourse.bass_utils import run_bass_kernel_spmd

    if _cached_nc is None:
        _cached_nc = build_nc()
    return run_bass_kernel_spmd(
        _cached_nc, in_maps, list(range(N_CORES)), trace=trace, **kwargs
    )


def gather(results):
    out = np.empty((N_TOK, D_OUT), dtype=np.float32)
    for c in range(N_CORES):
        ti, oi = divmod(c, OUT_SHARDS)
        out[ti * TOK_C : (ti + 1) * TOK_C, oi * OUT_C : (oi + 1) * OUT_C] = results[c][
            "out"
        ]
    return out


def kernel(x, weight, bias):
    res = run(prepare_in_maps(x, weight, bias), trace=False)
    return gather(res.results)


# revision 19
# speedup vs baseline: 1.0057x; 1.0057x over previous
"""Trainium2 kernel for BinaryLinear: out = x @ sign(clip(weight,-1,1)).T + bias.

Full shapes: x [8192, 4096] f32, weight [4096, 4096] f32, bias [4096] f32,
out [8192, 4096] f32. 8 NeuronCores, no collectives needed.

Design (measured on HW via neuron-profile):
  - Grid-shard tokens x out_features (2x4) across the 8 cores; each core
    computes a disjoint [4096, 1024] output tile; host slices inputs /
    stitches outputs.
  - Binarized weights are exactly +-1 (bf16-exact). x ships as bf16
    (~1.2e-3 rel error, gate is 2e-2). bf16 is the fastest matmul path:
    512-row moving streams at 1 col/cycle (216ns/matmul issue cadence =
    512 cycles @2.4GHz + 2.5ns NX dispatch). f32r pays a 187ns LDWEIGHTS
    that gates the cadence at 227ns; fp16 streams slower (235ns); fp8
    DoubleRow is only ~1.44x and would need hi/lo two passes (net loss).
    512 moving columns is the ISA max per matmul (s3d3_mm_num_elements).
  - Host packs x transposed+tiled so the contraction dim (in_features)
    lands on SBUF partitions with every DMA contiguous at line rate.
  - The PE busy floor is 2048 matmuls x 216ns = 442us; everything else
    is startup/stream/tail engineering:
    * Weights + first x chunks are woven across BOTH HW DGE queues
      (SP + Activation, ~170GB/s each) in first-need order with a greedy
      balance, in 256KB items. (Halving item granularity to 128KB was
      tried and is ~5us SLOWER overall: the doubled descriptor-write
      count puts recurring micro-stalls into the steady-state stream.)
    * Trickle phase: 3 token blocks round-robin over kb pairs consume
      weight k-blocks at 1.3us/kb vs ~1.1us/kb dual-queue arrival, so
      the weight slice becomes SBUF-resident with only ~3us of gaps.
    * Steady state: n-major per block; each PSUM bank is flushed
      (DVE adds bias while copying PSUM->SBUF) and DMA'd out the moment
      its accumulation closes, alternating output queues; x prefetch
      stays 2 blocks ahead on the SP queue.
    * The final bank flushes in two 256-col halves racing down both DMA
      queues to shorten the kernel tail.
  - Total ~462-470us (2.4GHz power mood): ~10us fixed runtime/DMA-start
    latency + 442us PE-bound compute + ~3us gaps + ~6us tail. The chip
    sometimes sits at a 2.0GHz package power state (~552us) - cadence
    259ns - which no kernel-side change can affect.
"""

import sys

if "/opt/trn_rl_repo" not in sys.path:
    sys.path.insert(0, "/opt/trn_rl_repo")

import ml_dtypes
import numpy as np

N_TOK, D_IN, D_OUT = 8192, 4096, 4096
TOK_SHARDS, OUT_SHARDS = 2, 4
N_CORES = TOK_SHARDS * OUT_SHARDS
TOK_C = N_TOK // TOK_SHARDS
OUT_C = D_OUT // OUT_SHARDS
MB = TOK_C // 128  # token blocks per core
KB = D_IN // 128  # contraction blocks
NF = 512  # matmul moving free dim (one fp32 PSUM bank, ISA max)
NB = OUT_C // NF  # PSUM banks per token block
XCH = 4  # x chunks per trickle block (8 kbs / 256KB each)
TRICKLE = 3  # token blocks interleaved with the weight stream at startup

_cached_nc = None


def build_nc():
    import concourse.bacc as bacc
    import concourse.mybir as mybir
    import concourse.tile as tile

    dt = mybir.dt
    mdt = dt.bfloat16

    nc = bacc.Bacc()
    xh_d = nc.dram_tensor("xh", [MB, 128, D_IN], mdt, kind="ExternalInput")
    wt_d = nc.dram_tensor("wt", [KB, 128, OUT_C], mdt, kind="ExternalInput")
    br_d = nc.dram_tensor("br", [128, OUT_C], dt.float32, kind="ExternalInput")
    out_d = nc.dram_tensor("out", [TOK_C, OUT_C], dt.float32, kind="ExternalOutput")

    with tile.TileContext(nc) as tc:
        with (
            tc.tile_pool(name="wts", bufs=1) as wpool,
            tc.tile_pool(name="bias", bufs=1) as bpool,
            tc.tile_pool(name="xin", bufs=6) as xpool,
            tc.tile_pool(name="outp", bufs=2) as opool,
            tc.tile_pool(name="psum", bufs=8, space="PSUM") as ppool,
        ):
            # --- greedy dual-queue DMA weave -------------------------------
            qclock = {"sync": 0.0, "scalar": 0.0}
            ITEM_US = 1.5  # ~256KB at ~170GB/s per active queue

            def enq(cost_units=1.0):
                q = min(qclock, key=qclock.get)
                qclock[q] += cost_units * ITEM_US
                return getattr(nc, q)

            wts = {}

            def load_w(kb):
                w = wpool.tile([128, OUT_C], mdt, name=f"wt{kb}", tag=f"wt{kb}")
                enq().dma_start(w[:], wt_d[kb])
                wts[kb] = w

            xt = {}
            xc_done = {}

            def x_tile(m):
                x = xpool.tile([128, D_IN], mdt, name=f"xh_{m}", tag="xh")
                xt[m] = x
                xc_done[m] = 0
                return x

            def load_x_chunk(m):
                c = xc_done[m]
                cw = D_IN // XCH
                lo, hi = c * cw, (c + 1) * cw
                enq().dma_start(xt[m][:, lo:hi], xh_d[m][:, lo:hi])
                xc_done[m] = c + 1

            def load_x_full(m, engine):
                x = x_tile(m)
                engine.dma_start(x[:], xh_d[m])
                xc_done[m] = XCH

            # Trickle DMA stream, first-need order with a one-chunk x lead.
            for m in range(TRICKLE):
                x_tile(m)
            load_x_chunk(0)
            load_w(0)
            load_w(1)
            load_x_chunk(1)
            load_x_chunk(2)
            for k0 in range(2, KB, 2):
                # request x chunk c when entering kb region 8c-4 (lead 4 kbs)
                if (k0 + 4) % 8 == 0:
                    for m in range(TRICKLE):
                        if xc_done[m] < XCH:
                            load_x_chunk(m)
                load_w(k0)
                load_w(k0 + 1)
            bias_s = bpool.tile([128, OUT_C], dt.float32, name="bias_s")
            enq(2.0).dma_start(bias_s[:], br_d[:])
            load_x_full(3, nc.sync)
            load_x_full(4, nc.sync)

            # --- compute ---------------------------------------------------
            def alloc_ps(m):
                return [
                    ppool.tile([128, NF], dt.float32, name=f"ps_{m}_{n}", tag="ps")
                    for n in range(NB)
                ]

            def mm(m, kb, n, ps):
                nc.tensor.matmul(
                    ps[n][:],
                    xt[m][:, kb * 128 : (kb + 1) * 128],
                    wts[kb][:, n * NF : (n + 1) * NF],
                    start=(kb == 0),
                    stop=(kb == KB - 1),
                )

            def flush_bank(m, n, ps, out_t, split=1):
                fw = NF // split
                for s in range(split):
                    lo, hi = n * NF + s * fw, n * NF + (s + 1) * fw
                    nc.vector.tensor_tensor(
                        out_t[:, lo:hi],
                        ps[n][:, s * fw : (s + 1) * fw],
                        bias_s[:, lo:hi],
                        mybir.AluOpType.add,
                    )
                    eng = nc.scalar if s % 2 == 0 else nc.sync
                    eng.dma_start(
                        out_d[m * 128 : (m + 1) * 128, lo:hi], out_t[:, lo:hi]
                    )

            # Trickle: 3 token blocks round-robin over kb pairs. Weight-kb
            # consumption 1.3us/kb vs dual-queue stream arrival ~1.1us/kb
            # (weights + x chunks + bias woven by first need), so the PE
            # tracks the stream with only pipeline-fill gaps at the start.
            tps = [alloc_ps(m) for m in range(TRICKLE)]
            for k0 in range(0, KB, 2):
                for m in range(TRICKLE):
                    for kb in (k0, k0 + 1):
                        for n in range(NB):
                            mm(m, kb, n, tps[m])
            for m in range(TRICKLE):
                out_t = opool.tile([128, OUT_C], dt.float32, name=f"o_{m}", tag="out")
                for n in range(NB):
                    flush_bank(m, n, tps[m], out_t)

            # Steady state: n-major per block, flush+store each PSUM bank as
            # soon as its accumulation closes. x prefetch stays 3 blocks out.
            for m in range(TRICKLE, MB):
                if m + 2 < MB:
                    load_x_full(m + 2, nc.sync)
                ps = alloc_ps(m)
                out_t = opool.tile([128, OUT_C], dt.float32, name=f"o_{m}", tag="out")
                last = m == MB - 1
                for n in range(NB):
                    if last and n == NB - 1:
                        # two independent 256-col accumulation groups in one
                        # PSUM bank: the left half closes a ~3.5us sweep early,
                        # so its flush+store overlap the right half's matmuls
                        # and only a 128KB store trails the final matmul.
                        for h in range(2):
                            cl = n * NF + h * 256
                            ch = cl + 256
                            pl, ph = h * 256, (h + 1) * 256
                            for kb in range(KB):
                                nc.tensor.matmul(
                                    ps[n][:, pl:ph],
                                    xt[m][:, kb * 128 : (kb + 1) * 128],
                                    wts[kb][:, cl:ch],
                                    start=(kb == 0),
                                    stop=(kb == KB - 1),
                                )
                            nc.vector.tensor_tensor(
                                out_t[:, cl:ch],
                                ps[n][:, pl:ph],
                                bias_s[:, cl:ch],
                                mybir.AluOpType.add,
                            )
                            eng = nc.scalar if h == 0 else nc.sync
                            eng.dma_start(
                                out_d[m * 128 : (m + 1) * 128, cl:ch],
                                out_t[:, cl:ch],
                            )
                    else:
                        for kb in range(KB):
                            mm(m, kb, n, ps)
                        flush_bank(m, n, ps, out_t)

    nc.compile()
    return nc


def _pack_x(a):
    """[TOK_C, D_IN] -> [MB, 128, D_IN] with layout [m, p, (kb t)]:
    packed[m, p, kb*128 + t] = a[m*128 + t, kb*128 + p]."""
    return np.ascontiguousarray(
        a.reshape(MB, 128, KB, 128).transpose(0, 3, 2, 1)
    ).reshape(MB, 128, D_IN)


def prepare_in_maps(x, weight, bias):
    x = np.asarray(x, dtype=np.float32)
    weight = np.asarray(weight, dtype=np.float32)
    bias = np.asarray(bias, dtype=np.float32)

    bw = np.where(weight >= 0, np.float32(1.0), np.float32(-1.0))

    wt_packs, bias_packs = [], []
    for oi in range(OUT_SHARDS):
        w_sh = bw[oi * OUT_C : (oi + 1) * OUT_C]  # [OUT_C, D_IN]
        wt = np.ascontiguousarray(w_sh.T).astype(ml_dtypes.bfloat16)
        wt_packs.append(wt.reshape(KB, 128, OUT_C))
        bias_packs.append(
            np.ascontiguousarray(
                np.broadcast_to(bias[oi * OUT_C : (oi + 1) * OUT_C], (128, OUT_C))
            )
        )

    xh_packs = []
    for ti in range(TOK_SHARDS):
        x_sh = x[ti * TOK_C : (ti + 1) * TOK_C].astype(ml_dtypes.bfloat16)
        xh_packs.append(_pack_x(x_sh))

    in_maps = []
    for c in range(N_CORES):
        ti, oi = divmod(c, OUT_SHARDS)
        in_maps.append(
            {"xh": xh_packs[ti], "wt": wt_packs[oi], "br": bias_packs[oi]}
        )
    return in_maps


def run(in_maps, trace=False, **kwargs):
    global _cached_nc
    from concourse.bass_utils import run_bass_kernel_spmd

    if _cached_nc is None:
        _cached_nc = build_nc()
    return run_bass_kernel_spmd(
        _cached_nc, in_maps, list(range(N_CORES)), trace=trace, **kwargs
    )


def gather(results):
    out = np.empty((N_TOK, D_OUT), dtype=np.float32)
    for c in range(N_CORES):
        ti, oi = divmod(c, OUT_SHARDS)
        out[ti * TOK_C : (ti + 1) * TOK_C, oi * OUT_C : (oi + 1) * OUT_C] = results[c][
            "out"
        ]
    return out


def kernel(x, weight, bias):
    res = run(prepare_in_maps(x, weight, bias), trace=False)
    return gather(res.results)


# revision 20
# speedup vs baseline: 1.2036x; 1.1968x over previous
"""Trainium2 kernel for BinaryLinear: out = x @ sign(clip(weight,-1,1)).T + bias.

Full shapes: x [8192, 4096] f32, weight [4096, 4096] f32, bias [4096] f32,
out [8192, 4096] f32. 8 NeuronCores, no collectives needed.

Design (measured on HW via neuron-profile):
  - Grid-shard tokens x out_features (2x4) across the 8 cores; each core
    computes a disjoint [4096, 1024] output tile; host slices inputs /
    stitches outputs.
  - Binarized weights are exactly +-1 (bf16-exact). x ships as bf16
    (~1.2e-3 rel error, gate is 2e-2). bf16 is the fastest matmul path:
    512-row moving streams at 1 col/cycle (216ns/matmul issue cadence =
    512 cycles @2.4GHz + 2.5ns NX dispatch). f32r pays a 187ns LDWEIGHTS
    that gates the cadence at 227ns; fp16 streams slower (235ns); fp8
    DoubleRow is only ~1.44x and would need hi/lo two passes (net loss).
    512 moving columns is the ISA max per matmul (s3d3_mm_num_elements).
  - Host packs x transposed+tiled so the contraction dim (in_features)
    lands on SBUF partitions with every DMA contiguous at line rate.
  - The PE busy floor is 2048 matmuls x 216ns = 442us; everything else
    is startup/stream/tail engineering:
    * Weights + first x chunks are woven across BOTH HW DGE queues
      (SP + Activation, ~170GB/s each) in first-need order with a greedy
      balance, in 256KB items. (Halving item granularity to 128KB was
      tried and is ~5us SLOWER overall: the doubled descriptor-write
      count puts recurring micro-stalls into the steady-state stream.)
    * Trickle phase: 3 token blocks round-robin over kb pairs consume
      weight k-blocks at 1.3us/kb vs ~1.1us/kb dual-queue arrival, so
      the weight slice becomes SBUF-resident with only ~3us of gaps.
    * Steady state: n-major per block; each PSUM bank is flushed
      (DVE adds bias while copying PSUM->SBUF) and DMA'd out the moment
      its accumulation closes, alternating output queues; x prefetch
      stays 2 blocks ahead on the SP queue.
    * The final PSUM bank runs as two independent 256-col accumulation
      groups: the left half closes a ~3.5us matmul sweep early so its
      flush+store overlap the right half's matmuls, leaving only a
      128KB store after the last matmul.
  - Total ~462us (2.4GHz power mood): ~10us fixed runtime/DMA-start
    latency + 442us PE-bound compute + ~1us real gaps + ~5us tail.
    (Profile-record drops make ~40 matmuls invisible; apparent 53ns
    "gaps" at 432ns start-to-start spacings are phantoms.) The chip
    sometimes sits at a 2.0GHz package power state (~554us) - cadence
    259ns - which no kernel-side change can affect. Shipping weights as
    fp8 + DVE upcast was tried and is ~3-5us slower: the per-kb cast
    latency lands in the trickle critical path.
"""

import sys

if "/opt/trn_rl_repo" not in sys.path:
    sys.path.insert(0, "/opt/trn_rl_repo")

import ml_dtypes
import numpy as np

N_TOK, D_IN, D_OUT = 8192, 4096, 4096
TOK_SHARDS, OUT_SHARDS = 2, 4
N_CORES = TOK_SHARDS * OUT_SHARDS
TOK_C = N_TOK // TOK_SHARDS
OUT_C = D_OUT // OUT_SHARDS
MB = TOK_C // 128  # token blocks per core
KB = D_IN // 128  # contraction blocks
NF = 512  # matmul moving free dim (one fp32 PSUM bank, ISA max)
NB = OUT_C // NF  # PSUM banks per token block
XCH = 4  # x chunks per trickle block (8 kbs / 256KB each)
TRICKLE = 3  # token blocks interleaved with the weight stream at startup

_cached_nc = None


def build_nc():
    import concourse.bacc as bacc
    import concourse.mybir as mybir
    import concourse.tile as tile

    dt = mybir.dt
    mdt = dt.bfloat16

    nc = bacc.Bacc()
    xh_d = nc.dram_tensor("xh", [MB, 128, D_IN], mdt, kind="ExternalInput")
    wt_d = nc.dram_tensor("wt", [KB, 128, OUT_C], mdt, kind="ExternalInput")
    br_d = nc.dram_tensor("br", [128, OUT_C], dt.float32, kind="ExternalInput")
    out_d = nc.dram_tensor("out", [TOK_C, OUT_C], dt.float32, kind="ExternalOutput")

    with tile.TileContext(nc) as tc:
        with (
            tc.tile_pool(name="wts", bufs=1) as wpool,
            tc.tile_pool(name="bias", bufs=1) as bpool,
            tc.tile_pool(name="xin", bufs=6) as xpool,
            tc.tile_pool(name="outp", bufs=2) as opool,
            tc.tile_pool(name="psum", bufs=8, space="PSUM") as ppool,
        ):
            # --- greedy dual-queue DMA weave -------------------------------
            qclock = {"sync": 0.0, "scalar": 0.0}
            ITEM_US = 1.5  # ~256KB at ~170GB/s per active queue

            def enq(cost_units=1.0):
                q = min(qclock, key=qclock.get)
                qclock[q] += cost_units * ITEM_US
                return getattr(nc, q)

            wts = {}

            def load_w(kb):
                w = wpool.tile([128, OUT_C], mdt, name=f"wt{kb}", tag=f"wt{kb}")
                enq().dma_start(w[:], wt_d[kb])
                wts[kb] = w

            xt = {}
            xc_done = {}

            def x_tile(m):
                x = xpool.tile([128, D_IN], mdt, name=f"xh_{m}", tag="xh")
                xt[m] = x
                xc_done[m] = 0
                return x

            def load_x_chunk(m):
                c = xc_done[m]
                cw = D_IN // XCH
                lo, hi = c * cw, (c + 1) * cw
                enq().dma_start(xt[m][:, lo:hi], xh_d[m][:, lo:hi])
                xc_done[m] = c + 1

            def load_x_full(m, engine):
                x = x_tile(m)
                engine.dma_start(x[:], xh_d[m])
                xc_done[m] = XCH

            # Trickle DMA stream, first-need order with a one-chunk x lead.
            for m in range(TRICKLE):
                x_tile(m)
            load_x_chunk(0)
            load_w(0)
            load_w(1)
            load_x_chunk(1)
            load_x_chunk(2)
            for k0 in range(2, KB, 2):
                # request x chunk c when entering kb region 8c-4 (lead 4 kbs)
                if (k0 + 4) % 8 == 0:
                    for m in range(TRICKLE):
                        if xc_done[m] < XCH:
                            load_x_chunk(m)
                load_w(k0)
                load_w(k0 + 1)
            bias_s = bpool.tile([128, OUT_C], dt.float32, name="bias_s")
            enq(2.0).dma_start(bias_s[:], br_d[:])
            load_x_full(3, nc.sync)
            load_x_full(4, nc.sync)

            # --- compute ---------------------------------------------------
            def alloc_ps(m):
                return [
                    ppool.tile([128, NF], dt.float32, name=f"ps_{m}_{n}", tag="ps")
                    for n in range(NB)
                ]

            def mm(m, kb, n, ps):
                nc.tensor.matmul(
                    ps[n][:],
                    xt[m][:, kb * 128 : (kb + 1) * 128],
                    wts[kb][:, n * NF : (n + 1) * NF],
                    start=(kb == 0),
                    stop=(kb == KB - 1),
                )

            def flush_bank(m, n, ps, out_t, split=1):
                fw = NF // split
                for s in range(split):
                    lo, hi = n * NF + s * fw, n * NF + (s + 1) * fw
                    nc.vector.tensor_tensor(
                        out_t[:, lo:hi],
                        ps[n][:, s * fw : (s + 1) * fw],
                        bias_s[:, lo:hi],
                        mybir.AluOpType.add,
                    )
                    eng = nc.scalar if s % 2 == 0 else nc.sync
                    eng.dma_start(
                        out_d[m * 128 : (m + 1) * 128, lo:hi], out_t[:, lo:hi]
                    )

            # Trickle: 3 token blocks round-robin over kb pairs. Weight-kb
            # consumption 1.3us/kb vs dual-queue stream arrival ~1.1us/kb
            # (weights + x chunks + bias woven by first need), so the PE
            # tracks the stream with only pipeline-fill gaps at the start.
            tps = [alloc_ps(m) for m in range(TRICKLE)]
            for k0 in range(0, KB, 2):
                for m in range(TRICKLE):
                    for kb in (k0, k0 + 1):
                        for n in range(NB):
                            mm(m, kb, n, tps[m])
            for m in range(TRICKLE):
                out_t = opool.tile([128, OUT_C], dt.float32, name=f"o_{m}", tag="out")
                for n in range(NB):
                    flush_bank(m, n, tps[m], out_t)

            # Steady state: n-major per block, flush+store each PSUM bank as
            # soon as its accumulation closes. x prefetch stays 3 blocks out.
            for m in range(TRICKLE, MB):
                if m + 2 < MB:
                    load_x_full(m + 2, nc.sync)
                ps = alloc_ps(m)
                out_t = opool.tile([128, OUT_C], dt.float32, name=f"o_{m}", tag="out")
                last = m == MB - 1
                for n in range(NB):
                    if last and n == NB - 1:
                        # two independent 256-col accumulation groups in one
                        # PSUM bank: the left half closes a ~3.5us sweep early,
                        # so its flush+store overlap the right half's matmuls
                        # and only a 128KB store trails the final matmul.
                        for h in range(2):
                            cl = n * NF + h * 256
                            ch = cl + 256
                            pl, ph = h * 256, (h + 1) * 256
                            for kb in range(KB):
                                nc.tensor.matmul(
                                    ps[n][:, pl:ph],
                                    xt[m][:, kb * 128 : (kb + 1) * 128],
                                    wts[kb][:, cl:ch],
                                    start=(kb == 0),
                                    stop=(kb == KB - 1),
                                )
                            nc.vector.tensor_tensor(
                                out_t[:, cl:ch],
                                ps[n][:, pl:ph],
                                bias_s[:, cl:ch],
                                mybir.AluOpType.add,
                            )
                            eng = nc.scalar if h == 0 else nc.sync
                            eng.dma_start(
                                out_d[m * 128 : (m + 1) * 128, cl:ch],
                                out_t[:, cl:ch],
                            )
                    else:
                        for kb in range(KB):
                            mm(m, kb, n, ps)
                        flush_bank(m, n, ps, out_t)

    nc.compile()
    return nc


def _pack_x(a):
    """[TOK_C, D_IN] -> [MB, 128, D_IN] with layout [m, p, (kb t)]:
    packed[m, p, kb*128 + t] = a[m*128 + t, kb*128 + p]."""
    return np.ascontiguousarray(
        a.reshape(MB, 128, KB, 128).transpose(0, 3, 2, 1)
    ).reshape(MB, 128, D_IN)


def prepare_in_maps(x, weight, bias):
    x = np.asarray(x, dtype=np.float32)
    weight = np.asarray(weight, dtype=np.float32)
    bias = np.asarray(bias, dtype=np.float32)

    bw = np.where(weight >= 0, np.float32(1.0), np.float32(-1.0))

    wt_packs, bias_packs = [], []
    for oi in range(OUT_SHARDS):
        w_sh = bw[oi * OUT_C : (oi + 1) * OUT_C]  # [OUT_C, D_IN]
        wt = np.ascontiguousarray(w_sh.T).astype(ml_dtypes.bfloat16)
        wt_packs.append(wt.reshape(KB, 128, OUT_C))
        bias_packs.append(
            np.ascontiguousarray(
                np.broadcast_to(bias[oi * OUT_C : (oi + 1) * OUT_C], (128, OUT_C))
            )
        )

    xh_packs = []
    for ti in range(TOK_SHARDS):
        x_sh = x[ti * TOK_C : (ti + 1) * TOK_C].astype(ml_dtypes.bfloat16)
        xh_packs.append(_pack_x(x_sh))

    in_maps = []
    for c in range(N_CORES):
        ti, oi = divmod(c, OUT_SHARDS)
        in_maps.append(
            {"xh": xh_packs[ti], "wt": wt_packs[oi], "br": bias_packs[oi]}
        )
    return in_maps


def run(in_maps, trace=False, **kwargs):
    global _cached_nc
    from concourse.bass_utils import run_bass_kernel_spmd

    if _cached_nc is None:
        _cached_nc = build_nc()
    return run_bass_kernel_spmd(
        _cached_nc, in_maps, list(range(N_CORES)), trace=trace, **kwargs
    )


def gather(results):
    out = np.empty((N_TOK, D_OUT), dtype=np.float32)
    for c in range(N_CORES):
        ti, oi = divmod(c, OUT_SHARDS)
        out[ti * TOK_C : (ti + 1) * TOK_C, oi * OUT_C : (oi + 1) * OUT_C] = results[c][
            "out"
        ]
    return out


def kernel(x, weight, bias):
    res = run(prepare_in_maps(x, weight, bias), trace=False)
    return gather(res.results)


# revision 21
# speedup vs baseline: 1.2045x; 1.0008x over previous
"""Trainium2 kernel for BinaryLinear: out = x @ sign(clip(weight,-1,1)).T + bias.

Full shapes: x [8192, 4096] f32, weight [4096, 4096] f32, bias [4096] f32,
out [8192, 4096] f32. 8 NeuronCores, no collectives needed.

Design (measured on HW via neuron-profile):
  - Grid-shard tokens x out_features (2x4) across the 8 cores; each core
    computes a disjoint [4096, 1024] output tile; host slices inputs /
    stitches outputs.
  - Binarized weights are exactly +-1 (bf16-exact). x ships as bf16
    (~1.2e-3 rel error, gate is 2e-2). bf16 is the fastest matmul path:
    512-row moving streams at 1 col/cycle (216ns/matmul issue cadence =
    512 cycles @2.4GHz + 2.5ns NX dispatch). f32r pays a 187ns LDWEIGHTS
    that gates the cadence at 227ns; fp16 streams slower (235ns); fp8
    DoubleRow is only ~1.44x and would need hi/lo two passes (net loss).
    512 moving columns is the ISA max per matmul (s3d3_mm_num_elements).
  - Host packs x transposed+tiled so the contraction dim (in_features)
    lands on SBUF partitions with every DMA contiguous at line rate.
  - The PE busy floor is 2048 matmuls x 216ns = 442us; everything else
    is startup/stream/tail engineering:
    * Weights + first x chunks are woven across BOTH HW DGE queues
      (SP + Activation, ~170GB/s each) in first-need order with a greedy
      balance, in 256KB items. (Halving item granularity to 128KB was
      tried and is ~5us SLOWER overall: the doubled descriptor-write
      count puts recurring micro-stalls into the steady-state stream.)
    * Trickle phase: 3 token blocks round-robin over kb pairs consume
      weight k-blocks at 1.3us/kb vs ~1.1us/kb dual-queue arrival, so
      the weight slice becomes SBUF-resident with only ~3us of gaps.
    * Steady state: n-major per block; each PSUM bank is flushed
      (DVE adds bias while copying PSUM->SBUF) and DMA'd out the moment
      its accumulation closes, alternating output queues; x prefetch
      stays 2 blocks ahead on the SP queue.
    * The final PSUM bank runs as two independent 256-col accumulation
      groups: the left half closes a ~3.5us matmul sweep early so its
      flush+store overlap the right half's matmuls, leaving only a
      128KB store after the last matmul.
  - Total ~462us (2.4GHz power mood): ~10us fixed runtime/DMA-start
    latency + 442us PE-bound compute + ~1us real gaps + ~5us tail.
    (Profile-record drops make ~40 matmuls invisible; apparent 53ns
    "gaps" at 432ns start-to-start spacings are phantoms.) The chip
    sometimes sits at a 2.0GHz package power state (~554us) - cadence
    259ns - which no kernel-side change can affect. Shipping weights as
    fp8 + DVE upcast was tried and is ~3-5us slower: the per-kb cast
    latency lands in the trickle critical path.
"""

import sys

if "/opt/trn_rl_repo" not in sys.path:
    sys.path.insert(0, "/opt/trn_rl_repo")

import ml_dtypes
import numpy as np

N_TOK, D_IN, D_OUT = 8192, 4096, 4096
TOK_SHARDS, OUT_SHARDS = 2, 4
N_CORES = TOK_SHARDS * OUT_SHARDS
TOK_C = N_TOK // TOK_SHARDS
OUT_C = D_OUT // OUT_SHARDS
MB = TOK_C // 128  # token blocks per core
KB = D_IN // 128  # contraction blocks
NF = 512  # matmul moving free dim (one fp32 PSUM bank, ISA max)
NB = OUT_C // NF  # PSUM banks per token block
XCH = 4  # x chunks per trickle block (8 kbs / 256KB each)
TRICKLE = 3  # token blocks interleaved with the weight stream at startup

_cached_nc = None


def build_nc():
    import concourse.bacc as bacc
    import concourse.mybir as mybir
    import concourse.tile as tile

    dt = mybir.dt
    mdt = dt.bfloat16

    nc = bacc.Bacc()
    xh_d = nc.dram_tensor("xh", [MB, 128, D_IN], mdt, kind="ExternalInput")
    wt_d = nc.dram_tensor("wt", [KB, 128, OUT_C], mdt, kind="ExternalInput")
    br_d = nc.dram_tensor("br", [128, OUT_C], dt.float32, kind="ExternalInput")
    out_d = nc.dram_tensor("out", [TOK_C, OUT_C], dt.float32, kind="ExternalOutput")

    with tile.TileContext(nc) as tc:
        with (
            tc.tile_pool(name="wts", bufs=1) as wpool,
            tc.tile_pool(name="bias", bufs=1) as bpool,
            tc.tile_pool(name="xin", bufs=6) as xpool,
            tc.tile_pool(name="outp", bufs=2) as opool,
            tc.tile_pool(name="psum", bufs=8, space="PSUM") as ppool,
        ):
            # --- greedy dual-queue DMA weave -------------------------------
            qclock = {"sync": 0.0, "scalar": 0.0}
            ITEM_US = 1.5  # ~256KB at ~170GB/s per active queue

            def enq(cost_units=1.0):
                q = min(qclock, key=qclock.get)
                qclock[q] += cost_units * ITEM_US
                return getattr(nc, q)

            wts = {}

            def load_w(kb):
                w = wpool.tile([128, OUT_C], mdt, name=f"wt{kb}", tag=f"wt{kb}")
                enq().dma_start(w[:], wt_d[kb])
                wts[kb] = w

            xt = {}
            xc_done = {}

            def x_tile(m):
                x = xpool.tile([128, D_IN], mdt, name=f"xh_{m}", tag="xh")
                xt[m] = x
                xc_done[m] = 0
                return x

            def load_x_chunk(m):
                c = xc_done[m]
                cw = D_IN // XCH
                lo, hi = c * cw, (c + 1) * cw
                enq().dma_start(xt[m][:, lo:hi], xh_d[m][:, lo:hi])
                xc_done[m] = c + 1

            def load_x_full(m, engine):
                x = x_tile(m)
                engine.dma_start(x[:], xh_d[m])
                xc_done[m] = XCH

            # Trickle DMA stream, first-need order with a one-chunk x lead.
            # Two 32KB bias slices lead the queues: they absorb the cold
            # DGE ramp (first item otherwise moves at ~70GB/s vs ~170 warm)
            # so the first x/weight blocks the PE waits on transfer warm.
            bias_s = bpool.tile([128, OUT_C], dt.float32, name="bias_s")
            nc.sync.dma_start(bias_s[:, 0:64], br_d[:, 0:64])
            nc.scalar.dma_start(bias_s[:, 64:128], br_d[:, 64:128])
            for q in qclock:
                qclock[q] += 0.125 * ITEM_US
            for m in range(TRICKLE):
                x_tile(m)
            load_x_chunk(0)
            load_w(0)
            load_w(1)
            load_x_chunk(1)
            load_x_chunk(2)
            for k0 in range(2, KB, 2):
                # request x chunk c when entering kb region 8c-4 (lead 4 kbs)
                if (k0 + 4) % 8 == 0:
                    for m in range(TRICKLE):
                        if xc_done[m] < XCH:
                            load_x_chunk(m)
                load_w(k0)
                load_w(k0 + 1)
            enq(1.75).dma_start(bias_s[:, 128:], br_d[:, 128:])
            load_x_full(3, nc.sync)
            load_x_full(4, nc.sync)

            # --- compute ---------------------------------------------------
            def alloc_ps(m):
                return [
                    ppool.tile([128, NF], dt.float32, name=f"ps_{m}_{n}", tag="ps")
                    for n in range(NB)
                ]

            def mm(m, kb, n, ps):
                nc.tensor.matmul(
                    ps[n][:],
                    xt[m][:, kb * 128 : (kb + 1) * 128],
                    wts[kb][:, n * NF : (n + 1) * NF],
                    start=(kb == 0),
                    stop=(kb == KB - 1),
                )

            def flush_bank(m, n, ps, out_t, split=1):
                fw = NF // split
                for s in range(split):
                    lo, hi = n * NF + s * fw, n * NF + (s + 1) * fw
                    nc.vector.tensor_tensor(
                        out_t[:, lo:hi],
                        ps[n][:, s * fw : (s + 1) * fw],
                        bias_s[:, lo:hi],
                        mybir.AluOpType.add,
                    )
                    eng = nc.scalar if s % 2 == 0 else nc.sync
                    eng.dma_start(
                        out_d[m * 128 : (m + 1) * 128, lo:hi], out_t[:, lo:hi]
                    )

            # Trickle: 3 token blocks round-robin over kb pairs. Weight-kb
            # consumption 1.3us/kb vs dual-queue stream arrival ~1.1us/kb
            # (weights + x chunks + bias woven by first need), so the PE
            # tracks the stream with only pipeline-fill gaps at the start.
            tps = [alloc_ps(m) for m in range(TRICKLE)]
            for k0 in range(0, KB, 2):
                for m in range(TRICKLE):
                    for kb in (k0, k0 + 1):
                        for n in range(NB):
                            mm(m, kb, n, tps[m])
            for m in range(TRICKLE):
                out_t = opool.tile([128, OUT_C], dt.float32, name=f"o_{m}", tag="out")
                for n in range(NB):
                    flush_bank(m, n, tps[m], out_t)

            # Steady state: n-major per block, flush+store each PSUM bank as
            # soon as its accumulation closes. x prefetch stays 3 blocks out.
            for m in range(TRICKLE, MB):
                if m + 2 < MB:
                    load_x_full(m + 2, nc.sync)
                ps = alloc_ps(m)
                out_t = opool.tile([128, OUT_C], dt.float32, name=f"o_{m}", tag="out")
                last = m == MB - 1
                for n in range(NB):
                    if last and n == NB - 1:
                        # two independent 256-col accumulation groups in one
                        # PSUM bank: the left half closes a ~3.5us sweep early,
                        # so its flush+store overlap the right half's matmuls
                        # and only a 128KB store trails the final matmul.
                        for h in range(2):
                            cl = n * NF + h * 256
                            ch = cl + 256
                            pl, ph = h * 256, (h + 1) * 256
                            for kb in range(KB):
                                nc.tensor.matmul(
                                    ps[n][:, pl:ph],
                                    xt[m][:, kb * 128 : (kb + 1) * 128],
                                    wts[kb][:, cl:ch],
                                    start=(kb == 0),
                                    stop=(kb == KB - 1),
                                )
                            nc.vector.tensor_tensor(
                                out_t[:, cl:ch],
                                ps[n][:, pl:ph],
                                bias_s[:, cl:ch],
                                mybir.AluOpType.add,
                            )
                            eng = nc.scalar if h == 0 else nc.sync
                            eng.dma_start(
                                out_d[m * 128 : (m + 1) * 128, cl:ch],
                                out_t[:, cl:ch],
                            )
                    else:
                        for kb in range(KB):
                            mm(m, kb, n, ps)
                        flush_bank(m, n, ps, out_t)

    nc.compile()
    return nc


def _pack_x(a):
    """[TOK_C, D_IN] -> [MB, 128, D_IN] with layout [m, p, (kb t)]:
    packed[m, p, kb*128 + t] = a[m*128 + t, kb*128 + p]."""
    return np.ascontiguousarray(
        a.reshape(MB, 128, KB, 128).transpose(0, 3, 2, 1)
    ).reshape(MB, 128, D_IN)


def prepare_in_maps(x, weight, bias):
    x = np.asarray(x, dtype=np.float32)
    weight = np.asarray(weight, dtype=np.float32)
    bias = np.asarray(bias, dtype=np.float32)

    bw = np.where(weight >= 0, np.float32(1.0), np.float32(-1.0))

    wt_packs, bias_packs = [], []
    for oi in range(OUT_SHARDS):
        w_sh = bw[oi * OUT_C : (oi + 1) * OUT_C]  # [OUT_C, D_IN]
        wt = np.ascontiguousarray(w_sh.T).astype(ml_dtypes.bfloat16)
        wt_packs.append(wt.reshape(KB, 128, OUT_C))
        bias_packs.append(
            np.ascontiguousarray(
                np.broadcast_to(bias[oi * OUT_C : (oi + 1) * OUT_C], (128, OUT_C))
            )
        )

    xh_packs = []
    for ti in range(TOK_SHARDS):
        x_sh = x[ti * TOK_C : (ti + 1) * TOK_C].astype(ml_dtypes.bfloat16)
        xh_packs.append(_pack_x(x_sh))

    in_maps = []
    for c in range(N_CORES):
        ti, oi = divmod(c, OUT_SHARDS)
        in_maps.append(
            {"xh": xh_packs[ti], "wt": wt_packs[oi], "br": bias_packs[oi]}
        )
    return in_maps


def run(in_maps, trace=False, **kwargs):
    global _cached_nc
    from concourse.bass_utils import run_bass_kernel_spmd

    if _cached_nc is None:
        _cached_nc = build_nc()
    return run_bass_kernel_spmd(
        _cached_nc, in_maps, list(range(N_CORES)), trace=trace, **kwargs
    )


def gather(results):
    out = np.empty((N_TOK, D_OUT), dtype=np.float32)
    for c in range(N_CORES):
        ti, oi = divmod(c, OUT_SHARDS)
        out[ti * TOK_C : (ti + 1) * TOK_C, oi * OUT_C : (oi + 1) * OUT_C] = results[c][
            "out"
        ]
    return out


def kernel(x, weight, bias):
    res = run(prepare_in_maps(x, weight, bias), trace=False)
    return gather(res.results)
